# revision 2
# baseline (speedup 1.0000x reference)
"""Trainium2 Bass kernel for nn_GateAttentionLayer (GAU-style gated attention).

Contract: kernel(**inputs) takes the FULL unsharded inputs (as produced by
setup_inputs()) and returns the full outputs (o [8,2048,1024] f32,
attn [8,2048,2048] f32), matching the reference's return structure.

Sharding: pure data-parallel over batch — one batch element per NeuronCore
(B=8, n_cores=8). Each core runs an identical program on its own slice.

Per-core math (all on-device except input/output transposes + bo add):
  A: x = gelu(queries @ Wqk + bqk) computed in [QK=128 part, L free] layout
     (fp32r matmuls); rope applied as x*CQ + swap(x)*SQ (+DQ) where swap is a
     PE permutation matmul and CQ/SQ/DQ are host-built tables with the
     per-branch affine (gq/betq) and the 1/sqrt(QK) score scale folded in.
  B: scores tile = q_rope^T(slice) @ k_rope (fp32r); softmax WITHOUT
     max-subtraction (scores bounded ~±10): ACT exp with fused row-sum
     (accum_out), reciprocal, scale -> attn rows DMA'd out in fp32.
  V: v = gelu(values @ Wv + bv) in [j part, v free] layout (fp32r; bv added
     via a K=1 ones x bv matmul accumulated into PSUM); ug = gelu(u @ Wu + bu)
     in [v part, i free] layout (bf16). Both spilled to DRAM in block layouts
     matching later consumption order.
  C/D per 1024-wide i-chunk: e_T = exp(scores_T) (bf16), av_T = v^T-blocks @
     e_T (bf16 matmuls, fp32 accum), out_T = ug_T * av_T, o_T = Wo^T-blocks @
     out_T, scaled by broadcast softmax reciprocal (exact, fp32) -> o rows.
"""

import os
import sys

for _p in ("/opt/trn_rl_repo", "/root/.axon_site/_ro/trn_rl_repo"):
    if os.path.isdir(_p) and _p not in sys.path:
        sys.path.insert(0, _p)

import numpy as np
import ml_dtypes
from contextlib import ExitStack

import concourse.bass as bass
import concourse.bacc as bacc
import concourse.tile as tile
from concourse import bass_utils, mybir

F32 = mybir.dt.float32
F32R = mybir.dt.float32r
BF16 = mybir.dt.bfloat16
AF = mybir.ActivationFunctionType
ts = bass.ts

B, L, D, QK, UV = 8, 2048, 1024, 128, 2048
P = 128
ROPE_BASE = 10000.0
N_CORES = 8


def build_program(shared_qk=True, use_dq=False, use_dk=False):
    nD, nL, nUV = D // P, L // P, UV // P
    LC = 512                      # l-chunk (f32r moving-operand limit)
    IC = min(1024, L)             # i-chunk for C/D (bf16 moving-operand limit)
    nLC, nIC = L // LC, L // IC
    icb = IC // LC

    nc = bacc.Bacc("TRN2", debug=False, num_devices=N_CORES)

    def din(name, shape, dt=F32):
        return nc.dram_tensor(name, shape, dt, kind="ExternalInput").ap()

    qT = din("qT", [D, L])
    vT = din("vT", [D, L])
    uT = din("uT", [D, L], BF16)
    Wqk = din("Wqk", [D, QK])
    Wv = din("Wv", [D, UV])
    Wu = din("Wu", [D, UV], BF16)
    Wo = din("Wo", [UV, D], BF16)
    bqk = din("bqk", [P, 1])
    bv = din("bv", [1, UV])
    bu = din("bu", [P, nUV])
    CQ = din("CQ", [P, L])
    SQ = din("SQ", [P, L])
    PSW = din("PSW", [P, P])
    ones1 = din("ones1", [1, P])
    if use_dq:
        DQ = din("DQ", [P, L])
    if not shared_qk:
        CK = din("CK", [P, L])
        SK = din("SK", [P, L])
        if use_dk:
            DK = din("DK", [P, L])

    attn_out = nc.dram_tensor("attn", [L, L], F32, kind="ExternalOutput").ap()
    oT_out = nc.dram_tensor("oT", [D, L], F32, kind="ExternalOutput").ap()

    with tile.TileContext(nc) as tc, ExitStack() as ctx:
        persist = ctx.enter_context(tc.tile_pool(name="persist", bufs=1))
        q_rope = persist.tile([P, L], F32R)
        k_rope = q_rope if shared_qk else persist.tile([P, L], F32R)
        r_col = persist.tile([P, nL], F32)
        r_bcast = persist.tile([P, L], F32)
        bv_sb = persist.tile([1, UV], F32R)
        bu_sb = persist.tile([P, nUV], F32)
        ones_sb = persist.tile([1, P], F32R)
        nc.sync.dma_start(bv_sb[:], bv[:].bitcast(F32R))
        nc.sync.dma_start(bu_sb[:], bu[:])
        nc.sync.dma_start(ones_sb[:], ones1[:].bitcast(F32R))

        dram = ctx.enter_context(tc.tile_pool(name="dram", bufs=1, space="DRAM"))
        v_spill = dram.tile([nL, nUV, P, P], BF16)       # [jt, vt, j, v]
        ug_spill = dram.tile([nUV, nIC, P, IC], BF16)    # [vt, ic, v, i]

        # ================= Phase A: qk projection + rope =================
        with ExitStack() as actx:
            apool = actx.enter_context(tc.tile_pool(name="apool", bufs=1))
            wqk_sb = apool.tile([P, D], F32R)
            psw_sb = apool.tile([P, P], F32R)
            bqk_sb = apool.tile([P, 1], F32)
            cq_sb = apool.tile([P, L], F32)
            sq_sb = apool.tile([P, L], F32)
            for t in range(nD):
                nc.sync.dma_start(wqk_sb[:, ts(t, P)], Wqk[ts(t, P), :].bitcast(F32R))
            nc.sync.dma_start(psw_sb[:], PSW[:].bitcast(F32R))
            nc.sync.dma_start(bqk_sb[:], bqk[:])
            nc.sync.dma_start(cq_sb[:], CQ[:])
            nc.sync.dma_start(sq_sb[:], SQ[:])
            if use_dq:
                dq_sb = apool.tile([P, L], F32)
                nc.sync.dma_start(dq_sb[:], DQ[:])
            if not shared_qk:
                ck_sb = apool.tile([P, L], F32)
                sk_sb = apool.tile([P, L], F32)
                nc.sync.dma_start(ck_sb[:], CK[:])
                nc.sync.dma_start(sk_sb[:], SK[:])
                if use_dk:
                    dk_sb = apool.tile([P, L], F32)
                    nc.sync.dma_start(dk_sb[:], DK[:])

            aq = actx.enter_context(tc.tile_pool(name="aq", bufs=2 * nD))
            aps = actx.enter_context(tc.tile_pool(name="aps", bufs=2, space="PSUM"))
            axs = actx.enter_context(tc.tile_pool(name="axs", bufs=2, space="PSUM"))
            atmp = actx.enter_context(tc.tile_pool(name="atmp", bufs=6))

            for c in range(nLC):
                qts = []
                for t in range(nD):
                    qt = aq.tile([P, LC], F32R, tag="qt")
                    nc.sync.dma_start(qt[:], qT[ts(t, P), ts(c, LC)].bitcast(F32R))
                    qts.append(qt)
                g_ps = aps.tile([P, LC], F32)
                for t in range(nD):
                    nc.tensor.matmul(g_ps[:], wqk_sb[:, ts(t, P)], qts[t][:],
                                     start=(t == 0), stop=(t == nD - 1))
                x_sb = atmp.tile([P, LC], F32R, tag="x")
                nc.scalar.activation(x_sb[:], g_ps[:], AF.Gelu_apprx_tanh,
                                     bias=bqk_sb[:])
                xs_ps = axs.tile([P, LC], F32)
                nc.tensor.matmul(xs_ps[:], psw_sb[:], x_sb[:],
                                 start=True, stop=True)

                def rope_combine(dst, c_t, s_t, d_t):
                    ta = atmp.tile([P, LC], F32, tag="ta")
                    tb = atmp.tile([P, LC], F32, tag="tb")
                    nc.vector.tensor_mul(ta[:], x_sb[:].bitcast(F32), c_t[:, ts(c, LC)])
                    nc.vector.tensor_mul(tb[:], xs_ps[:], s_t[:, ts(c, LC)])
                    if d_t is None:
                        nc.vector.tensor_add(dst[:, ts(c, LC)], ta[:], tb[:])
                    else:
                        tc2 = atmp.tile([P, LC], F32, tag="tc")
                        nc.vector.tensor_add(tc2[:], ta[:], tb[:])
                        nc.vector.tensor_add(dst[:, ts(c, LC)], tc2[:],
                                             d_t[:, ts(c, LC)])

                rope_combine(q_rope, cq_sb, sq_sb, dq_sb if use_dq else None)
                if not shared_qk:
                    rope_combine(k_rope, ck_sb, sk_sb, dk_sb if use_dk else None)

        # ============ Phase B: scores + softmax + attn out ===============
        with ExitStack() as bctx:
            bps = bctx.enter_context(tc.tile_pool(name="bps", bufs=2, space="PSUM"))
            be = bctx.enter_context(tc.tile_pool(name="be", bufs=2))
            battn = bctx.enter_context(tc.tile_pool(name="battn", bufs=2))
            bsum = bctx.enter_context(tc.tile_pool(name="bsum", bufs=4))
            for t in range(nL):
                s_ps = bps.tile([P, L], F32)
                for jc in range(nLC):
                    nc.tensor.matmul(s_ps[:, ts(jc, LC)], q_rope[:, ts(t, P)],
                                     k_rope[:, ts(jc, LC)], start=True, stop=True)
                e_sb = be.tile([P, L], F32)
                sum_sb = bsum.tile([P, 1], F32)
                nc.scalar.activation(e_sb[:], s_ps[:], AF.Exp,
                                     accum_out=sum_sb[:])
                nc.vector.reciprocal(r_col[:, t:t + 1], sum_sb[:])
                a_sb = battn.tile([P, L], F32)
                nc.vector.tensor_scalar_mul(a_sb[:], e_sb[:], r_col[:, t:t + 1])
                nc.sync.dma_start(attn_out[ts(t, P), :], a_sb[:])

        # r_row / r_bcast
        with ExitStack() as rctx:
            r_row = persist.tile([1, L], F32R)
            for t in range(nL):
                nc.sync.dma_start(r_row[:, ts(t, P)], r_col[:, t:t + 1].bitcast(F32R))
            rbps = rctx.enter_context(tc.tile_pool(name="rbps", bufs=1, space="PSUM"))
            rb_ps = rbps.tile([P, L], F32)
            for c in range(nLC):
                nc.tensor.matmul(rb_ps[:, ts(c, LC)], ones_sb[:],
                                 r_row[:, ts(c, LC)], start=True, stop=True)
            nc.scalar.copy(r_bcast[:], rb_ps[:])

        # ================= Phase V: v (spill) + ug (spill) ================
        with ExitStack() as vctx:
            vtp = vctx.enter_context(tc.tile_pool(name="vtp", bufs=1))
            vt_sb = [vtp.tile([P, L], F32R, name=f"vt_sb{t}") for t in range(nD)]
            for t in range(nD):
                nc.sync.dma_start(vt_sb[t][:], vT[ts(t, P), :].bitcast(F32R))
            wvp = vctx.enter_context(tc.tile_pool(name="wvp", bufs=nD + 2))
            vps = vctx.enter_context(tc.tile_pool(name="vps", bufs=3, space="PSUM"))
            vsb = vctx.enter_context(tc.tile_pool(name="vsb", bufs=4))
            for vc in range(UV // LC):
                wvs = []
                for t in range(nD):
                    wv = wvp.tile([P, LC], F32R, tag="wv")
                    nc.sync.dma_start(wv[:], Wv[ts(t, P), ts(vc, LC)].bitcast(F32R))
                    wvs.append(wv)
                for jt in range(nL):
                    v_ps = vps.tile([P, LC], F32)
                    for t in range(nD):
                        nc.tensor.matmul(v_ps[:], vt_sb[t][:, ts(jt, P)], wvs[t][:],
                                         start=(t == 0), stop=False)
                    nc.tensor.matmul(v_ps[:], ones_sb[:], bv_sb[:, ts(vc, LC)],
                                     start=False, stop=True)
                    v_bf = vsb.tile([P, LC], BF16, tag="vbf")
                    nc.scalar.activation(v_bf[:], v_ps[:], AF.Gelu_apprx_tanh)
                    for k in range(LC // P):
                        nc.sync.dma_start(v_spill[jt, vc * (LC // P) + k],
                                          v_bf[:, ts(k, P)])

            wup = vctx.enter_context(tc.tile_pool(name="wup", bufs=1))
            wu_sb = [wup.tile([P, UV], BF16, name=f"wu_sb{t}") for t in range(nD)]
            for t in range(nD):
                nc.sync.dma_start(wu_sb[t][:], Wu[ts(t, P), :])
            utp = vctx.enter_context(tc.tile_pool(name="utp", bufs=nD + 2))
            ugps = vctx.enter_context(tc.tile_pool(name="ugps", bufs=2, space="PSUM"))
            ugsb = vctx.enter_context(tc.tile_pool(name="ugsb", bufs=4))
            for ic in range(nIC):
                uts = []
                for t in range(nD):
                    ut = utp.tile([P, IC], BF16, tag="ut")
                    nc.sync.dma_start(ut[:], uT[ts(t, P), ts(ic, IC)])
                    uts.append(ut)
                for vt in range(nUV):
                    ug_ps = ugps.tile([P, IC], F32)
                    for t in range(nD):
                        for k in range(icb):
                            nc.tensor.matmul(ug_ps[:, ts(k, LC)],
                                             wu_sb[t][:, ts(vt, P)],
                                             uts[t][:, ts(k, LC)],
                                             start=(t == 0), stop=(t == nD - 1))
                    ug_bf = ugsb.tile([P, IC], BF16, tag="ug")
                    nc.scalar.activation(ug_bf[:], ug_ps[:], AF.Gelu_apprx_tanh,
                                         bias=bu_sb[:, vt:vt + 1])
                    nc.sync.dma_start(ug_spill[vt, ic], ug_bf[:])

        # ================= Phase C/D: e_T, AV, out, o =====================
        with ExitStack() as cctx:
            wop = cctx.enter_context(tc.tile_pool(name="wop", bufs=1))
            wo_sb = [wop.tile([P, D], BF16, name=f"wo_sb{vt}") for vt in range(nUV)]
            for vt in range(nUV):
                nc.sync.dma_start(wo_sb[vt][:], Wo[ts(vt, P), :])
            stps = cctx.enter_context(tc.tile_pool(name="stps", bufs=1, space="PSUM"))
            ep = cctx.enter_context(tc.tile_pool(name="ep", bufs=nL + 2))
            vstr = cctx.enter_context(tc.tile_pool(name="vstr", bufs=32))
            ugstr = cctx.enter_context(tc.tile_pool(name="ugstr", bufs=4))
            avps = cctx.enter_context(tc.tile_pool(name="avps", bufs=2, space="PSUM"))
            outp = cctx.enter_context(tc.tile_pool(name="outp", bufs=nUV + 2))
            ops_ = cctx.enter_context(tc.tile_pool(name="ops", bufs=1, space="PSUM"))
            osb = cctx.enter_context(tc.tile_pool(name="osb", bufs=3))

            for ic in range(nIC):
                e_tiles = []
                for jt in range(nL):
                    st_ps = stps.tile([P, IC], F32)
                    for k in range(icb):
                        nc.tensor.matmul(st_ps[:, ts(k, LC)], k_rope[:, ts(jt, P)],
                                         q_rope[:, ts(ic * icb + k, LC)],
                                         start=True, stop=True)
                    e_bf = ep.tile([P, IC], BF16, tag="ebf")
                    nc.scalar.activation(e_bf[:], st_ps[:], AF.Exp)
                    e_tiles.append(e_bf)
                out_tiles = []
                for vt in range(nUV):
                    av_ps = avps.tile([P, IC], F32)
                    vtiles = []
                    for jt in range(nL):
                        vtile = vstr.tile([P, P], BF16, tag="vstr")
                        nc.sync.dma_start(vtile[:], v_spill[jt, vt])
                        vtiles.append(vtile)
                    for jt in range(nL):
                        for k in range(icb):
                            nc.tensor.matmul(av_ps[:, ts(k, LC)], vtiles[jt][:],
                                             e_tiles[jt][:, ts(k, LC)],
                                             start=(jt == 0), stop=(jt == nL - 1))
                    ug_bf = ugstr.tile([P, IC], BF16, tag="ugs")
                    nc.sync.dma_start(ug_bf[:], ug_spill[vt, ic])
                    out_bf = outp.tile([P, IC], BF16, tag="out")
                    nc.vector.tensor_mul(out_bf[:], av_ps[:], ug_bf[:])
                    out_tiles.append(out_bf)
                for dt in range(nD):
                    o_ps = ops_.tile([P, IC], F32)
                    for vt in range(nUV):
                        for k in range(icb):
                            nc.tensor.matmul(o_ps[:, ts(k, LC)],
                                             wo_sb[vt][:, ts(dt, P)],
                                             out_tiles[vt][:, ts(k, LC)],
                                             start=(vt == 0), stop=(vt == nUV - 1))
                    o_sb = osb.tile([P, IC], F32, tag="osb")
                    nc.vector.tensor_mul(o_sb[:], o_ps[:], r_bcast[:, ts(ic, IC)])
                    nc.sync.dma_start(oT_out[ts(dt, P), ts(ic, IC)], o_sb[:])

    nc.compile()
    return nc


def prep_inputs(u, queries, keys, values, Wqk, bqk, gq, betq, gk, betk,
                Wv, bv, Wu, bu, Wo, bo, shared_qk, use_dq, use_dk):
    scale = float(QK) ** -0.25
    inv = 1.0 / (ROPE_BASE ** (np.arange(0, QK, 2, dtype=np.float64) / QK))
    t = np.arange(L, dtype=np.float64)
    f = t[None, :] * inv[:, None]            # [QK/2, L]
    c, s = np.cos(f), np.sin(f)

    def mk_tables(g, bet):
        g64, b64 = np.asarray(g, np.float64), np.asarray(bet, np.float64)
        C = np.empty((P, L))
        S = np.empty((P, L))
        Dt = np.empty((P, L))
        C[0::2] = g64[0::2, None] * c
        C[1::2] = g64[1::2, None] * c
        S[0::2] = -g64[1::2, None] * s
        S[1::2] = g64[0::2, None] * s
        Dt[0::2] = b64[0::2, None] * c - b64[1::2, None] * s
        Dt[1::2] = b64[0::2, None] * s + b64[1::2, None] * c
        return ((C * scale).astype(np.float32), (S * scale).astype(np.float32),
                (Dt * scale).astype(np.float32))

    CQ, SQ, DQ = mk_tables(gq, betq)
    PSW = np.zeros((P, P), np.float32)
    for i in range(0, P, 2):
        PSW[i, i + 1] = 1.0
        PSW[i + 1, i] = 1.0

    static = {
        "Wqk": np.ascontiguousarray(Wqk, np.float32),
        "Wv": np.ascontiguousarray(Wv, np.float32),
        "Wu": np.ascontiguousarray(Wu).astype(ml_dtypes.bfloat16),
        "Wo": np.ascontiguousarray(Wo).astype(ml_dtypes.bfloat16),
        "bqk": np.asarray(bqk, np.float32).reshape(P, 1),
        "bv": np.asarray(bv, np.float32).reshape(1, UV),
        "bu": np.ascontiguousarray(np.asarray(bu, np.float32).reshape(UV // P, P).T),
        "CQ": CQ, "SQ": SQ, "PSW": PSW,
        "ones1": np.ones((1, P), np.float32),
    }
    if use_dq:
        static["DQ"] = DQ
    if not shared_qk:
        CK, SK, DK = mk_tables(gk, betk)
        static["CK"], static["SK"] = CK, SK
        if use_dk:
            static["DK"] = DK

    in_maps = []
    for b in range(B):
        m = dict(static)
        m["qT"] = np.ascontiguousarray(queries[b].T, np.float32)
        m["vT"] = np.ascontiguousarray(values[b].T, np.float32)
        m["uT"] = np.ascontiguousarray(u[b].T).astype(ml_dtypes.bfloat16)
        in_maps.append(m)
    return in_maps


_cache = {}


def _get_program(flags):
    if flags not in _cache:
        _cache[flags] = build_program(*flags)
    return _cache[flags]


def kernel(u, queries, keys, values, Wqk, bqk, gq, betq, gk, betk,
           Wv, bv, Wu, bu, Wo, bo, _trace=False, _tmpdir=None):
    shared_qk = (np.array_equal(np.asarray(gq), np.asarray(gk))
                 and np.array_equal(np.asarray(betq), np.asarray(betk)))
    use_dq = bool(np.any(np.asarray(betq)))
    use_dk = (not shared_qk) and bool(np.any(np.asarray(betk)))

    nc = _get_program((shared_qk, use_dq, use_dk))
    in_maps = prep_inputs(u, queries, keys, values, Wqk, bqk, gq, betq, gk,
                          betk, Wv, bv, Wu, bu, Wo, bo,
                          shared_qk, use_dq, use_dk)
    res = bass_utils.run_bass_kernel_spmd(
        nc, in_maps, core_ids=list(range(N_CORES)),
        trace=_trace, tmpdir=_tmpdir)

    bo32 = np.asarray(bo, np.float32)
    o = np.empty((B, L, D), np.float32)
    attn = np.empty((B, L, L), np.float32)
    for b in range(B):
        o[b] = res.results[b]["oT"].T + bo32
        attn[b] = res.results[b]["attn"]
    if _trace:
        kernel._last_exec_time_ns = res.exec_time_ns
    return o, attn


# revision 3
# speedup vs baseline: 1.0703x; 1.0703x over previous
"""Trainium2 Bass kernel for nn_GateAttentionLayer (GAU-style gated attention).

Contract: kernel(**inputs) takes the FULL unsharded inputs (as produced by
setup_inputs()) and returns the full outputs (o [8,2048,1024] f32,
attn [8,2048,2048] f32), matching the reference's return structure.

Sharding: pure data-parallel over batch — one batch element per NeuronCore
(B=8, n_cores=8). Each core runs an identical program on its own slice.

Per-core math (all on-device except input/output transposes + bo add):
  A: x = gelu(queries @ Wqk + bqk) computed in [QK=128 part, L free] layout
     (fp32r matmuls); rope applied as x*CQ + swap(x)*SQ (+DQ) where swap is a
     PE permutation matmul and CQ/SQ/DQ are host-built tables with the
     per-branch affine (gq/betq) and the 1/sqrt(QK) score scale folded in.
  B: scores tile = q_rope^T(slice) @ k_rope (fp32r); softmax WITHOUT
     max-subtraction (scores bounded ~±10): ACT exp with fused row-sum
     (accum_out), reciprocal, scale -> attn rows DMA'd out in fp32.
  V: v = gelu(values @ Wv + bv) in [j part, v free] layout (fp32r; bv added
     via a K=1 ones x bv matmul accumulated into PSUM); ug = gelu(u @ Wu + bu)
     in [v part, i free] layout (bf16). Both spilled to DRAM in block layouts
     matching later consumption order.
  C/D per 1024-wide i-chunk: e_T = exp(scores_T) (bf16), av_T = v^T-blocks @
     e_T (bf16 matmuls, fp32 accum), out_T = ug_T * av_T, o_T = Wo^T-blocks @
     out_T, scaled by broadcast softmax reciprocal (exact, fp32) -> o rows.
"""

import os
import sys

for _p in ("/opt/trn_rl_repo", "/root/.axon_site/_ro/trn_rl_repo"):
    if os.path.isdir(_p) and _p not in sys.path:
        sys.path.insert(0, _p)

import numpy as np
import ml_dtypes
from contextlib import ExitStack

import concourse.bass as bass
import concourse.bacc as bacc
import concourse.tile as tile
from concourse import bass_utils, mybir

F32 = mybir.dt.float32
F32R = mybir.dt.float32r
BF16 = mybir.dt.bfloat16
AF = mybir.ActivationFunctionType
ts = bass.ts

B, L, D, QK, UV = 8, 2048, 1024, 128, 2048
P = 128
ROPE_BASE = 10000.0
N_CORES = 8


def build_program(shared_qk=True, use_dq=False, use_dk=False):
    nD, nL, nUV = D // P, L // P, UV // P
    LC = 512                      # l-chunk (f32r moving-operand limit)
    IC = min(1024, L)             # i-chunk for C/D (bf16 moving-operand limit)
    nLC, nIC = L // LC, L // IC
    icb = IC // LC

    nc = bacc.Bacc("TRN2", debug=False, num_devices=N_CORES)

    def din(name, shape, dt=F32):
        return nc.dram_tensor(name, shape, dt, kind="ExternalInput").ap()

    qT = din("qT", [D, L])
    vT = din("vT", [D, L])
    uT = din("uT", [D, L], BF16)
    Wqk = din("Wqk", [D, QK])
    Wv = din("Wv", [D, UV])
    Wu = din("Wu", [D, UV], BF16)
    Wo = din("Wo", [UV, D], BF16)
    bqk = din("bqk", [P, 1])
    bv = din("bv", [1, UV])
    bu = din("bu", [P, nUV])
    CQ = din("CQ", [P, L])
    SQ = din("SQ", [P, L])
    PSW = din("PSW", [P, P])
    ones1 = din("ones1", [1, P])
    if use_dq:
        DQ = din("DQ", [P, L])
    if not shared_qk:
        CK = din("CK", [P, L])
        SK = din("SK", [P, L])
        if use_dk:
            DK = din("DK", [P, L])

    attn_out = nc.dram_tensor("attn", [L, L], F32, kind="ExternalOutput").ap()
    oT_out = nc.dram_tensor("oT", [D, L], F32, kind="ExternalOutput").ap()

    with tile.TileContext(nc) as tc, ExitStack() as ctx:
        persist = ctx.enter_context(tc.tile_pool(name="persist", bufs=1))
        q_rope = persist.tile([P, L], F32R)
        k_rope = q_rope if shared_qk else persist.tile([P, L], F32R)
        r_col = persist.tile([P, nL], F32)
        r_bcast = persist.tile([P, L], F32)
        bv_sb = persist.tile([1, UV], F32R)
        bu_sb = persist.tile([P, nUV], F32)
        ones_sb = persist.tile([1, P], F32R)
        nc.sync.dma_start(bv_sb[:], bv[:].bitcast(F32R))
        nc.sync.dma_start(bu_sb[:], bu[:])
        nc.sync.dma_start(ones_sb[:], ones1[:].bitcast(F32R))

        dram = ctx.enter_context(tc.tile_pool(name="dram", bufs=1, space="DRAM"))
        v_spill = dram.tile([nUV, P, L], BF16)           # [vt, j%P, jt*P+v']
        ug_spill = dram.tile([nUV, nIC, P, IC], BF16)    # [vt, ic, v, i]

        # ================= Phase A: qk projection + rope =================
        with ExitStack() as actx:
            apool = actx.enter_context(tc.tile_pool(name="apool", bufs=1))
            wqk_sb = apool.tile([P, D], F32R)
            psw_sb = apool.tile([P, P], F32R)
            bqk_sb = apool.tile([P, 1], F32)
            cq_sb = apool.tile([P, L], F32)
            sq_sb = apool.tile([P, L], F32)
            for t in range(nD):
                nc.sync.dma_start(wqk_sb[:, ts(t, P)], Wqk[ts(t, P), :].bitcast(F32R))
            nc.sync.dma_start(psw_sb[:], PSW[:].bitcast(F32R))
            nc.sync.dma_start(bqk_sb[:], bqk[:])
            if use_dq:
                dq_sb = apool.tile([P, L], F32)
                nc.sync.dma_start(dq_sb[:], DQ[:])
            if not shared_qk:
                ck_sb = apool.tile([P, L], F32)
                sk_sb = apool.tile([P, L], F32)
                nc.sync.dma_start(ck_sb[:], CK[:])
                nc.sync.dma_start(sk_sb[:], SK[:])
                if use_dk:
                    dk_sb = apool.tile([P, L], F32)
                    nc.sync.dma_start(dk_sb[:], DK[:])

            aq = actx.enter_context(tc.tile_pool(name="aq", bufs=2 * nD))
            aps = actx.enter_context(tc.tile_pool(name="aps", bufs=2, space="PSUM"))
            axs = actx.enter_context(tc.tile_pool(name="axs", bufs=2, space="PSUM"))
            atmp = actx.enter_context(tc.tile_pool(name="atmp", bufs=6))

            for c in range(nLC):
                qts = []
                for t in range(nD):
                    qt = aq.tile([P, LC], F32R, tag="qt")
                    nc.sync.dma_start(qt[:], qT[ts(t, P), ts(c, LC)].bitcast(F32R))
                    qts.append(qt)
                if c == 0:
                    # tables are needed only after the first gelu; keep them
                    # behind the first chunk's loads in the sync queue
                    nc.sync.dma_start(cq_sb[:], CQ[:])
                    nc.sync.dma_start(sq_sb[:], SQ[:])
                g_ps = aps.tile([P, LC], F32)
                for t in range(nD):
                    nc.tensor.matmul(g_ps[:], wqk_sb[:, ts(t, P)], qts[t][:],
                                     start=(t == 0), stop=(t == nD - 1))
                x_sb = atmp.tile([P, LC], F32R, tag="x")
                nc.scalar.activation(x_sb[:], g_ps[:], AF.Gelu_apprx_tanh,
                                     bias=bqk_sb[:])
                xs_ps = axs.tile([P, LC], F32)
                nc.tensor.matmul(xs_ps[:], psw_sb[:], x_sb[:],
                                 start=True, stop=True)

                def rope_combine(dst, c_t, s_t, d_t):
                    ta = atmp.tile([P, LC], F32, tag="ta")
                    tb = atmp.tile([P, LC], F32, tag="tb")
                    nc.vector.tensor_mul(ta[:], x_sb[:].bitcast(F32), c_t[:, ts(c, LC)])
                    nc.vector.tensor_mul(tb[:], xs_ps[:], s_t[:, ts(c, LC)])
                    if d_t is None:
                        nc.vector.tensor_add(dst[:, ts(c, LC)], ta[:], tb[:])
                    else:
                        tc2 = atmp.tile([P, LC], F32, tag="tc")
                        nc.vector.tensor_add(tc2[:], ta[:], tb[:])
                        nc.vector.tensor_add(dst[:, ts(c, LC)], tc2[:],
                                             d_t[:, ts(c, LC)])

                rope_combine(q_rope, cq_sb, sq_sb, dq_sb if use_dq else None)
                if not shared_qk:
                    rope_combine(k_rope, ck_sb, sk_sb, dk_sb if use_dk else None)

        # ============ Phase B: scores + softmax + attn out ===============
        with ExitStack() as bctx:
            bps = bctx.enter_context(tc.tile_pool(name="bps", bufs=2, space="PSUM"))
            be = bctx.enter_context(tc.tile_pool(name="be", bufs=2))
            battn = bctx.enter_context(tc.tile_pool(name="battn", bufs=2))
            bsum = bctx.enter_context(tc.tile_pool(name="bsum", bufs=4))
            HB = L // 2
            for t in range(nL):
                e_sb = be.tile([P, L], F32)
                sums = []
                for h in range(2):
                    s_ps = bps.tile([P, HB], F32, tag="s_ps")
                    for jc in range(HB // LC):
                        nc.tensor.matmul(s_ps[:, ts(jc, LC)], q_rope[:, ts(t, P)],
                                         k_rope[:, ts(h * (HB // LC) + jc, LC)],
                                         start=True, stop=True)
                    sum_sb = bsum.tile([P, 1], F32, tag="sum")
                    nc.scalar.activation(e_sb[:, ts(h, HB)], s_ps[:], AF.Exp,
                                         accum_out=sum_sb[:])
                    sums.append(sum_sb)
                tot_sb = bsum.tile([P, 1], F32, tag="tot")
                nc.vector.tensor_add(tot_sb[:], sums[0][:], sums[1][:])
                nc.vector.reciprocal(r_col[:, t:t + 1], tot_sb[:])
                a_sb = battn.tile([P, L], F32)
                nc.vector.tensor_scalar_mul(a_sb[:], e_sb[:], r_col[:, t:t + 1])
                nc.sync.dma_start(attn_out[ts(t, P), :], a_sb[:])

        # r_row / r_bcast
        with ExitStack() as rctx:
            r_row = persist.tile([1, L], F32R)
            for t in range(nL):
                nc.sync.dma_start(r_row[:, ts(t, P)], r_col[:, t:t + 1].bitcast(F32R))
            rbps = rctx.enter_context(tc.tile_pool(name="rbps", bufs=1, space="PSUM"))
            rb_ps = rbps.tile([P, L], F32)
            for c in range(nLC):
                nc.tensor.matmul(rb_ps[:, ts(c, LC)], ones_sb[:],
                                 r_row[:, ts(c, LC)], start=True, stop=True)
            nc.scalar.copy(r_bcast[:], rb_ps[:])

        # ================= Phase V: ug (spill), then v (spill) ============
        with ExitStack() as uctx:
            wup = uctx.enter_context(tc.tile_pool(name="wup", bufs=1))
            wu_sb = [wup.tile([P, UV], BF16, name=f"wu_sb{t}") for t in range(nD)]
            for t in range(nD):
                nc.sync.dma_start(wu_sb[t][:], Wu[ts(t, P), :])
            utp = uctx.enter_context(tc.tile_pool(name="utp", bufs=2 * nD + 2))
            ugps = uctx.enter_context(tc.tile_pool(name="ugps", bufs=2, space="PSUM"))
            ugsb = uctx.enter_context(tc.tile_pool(name="ugsb", bufs=4))
            for ic in range(nIC):
                uts = []
                for t in range(nD):
                    ut = utp.tile([P, IC], BF16, tag="ut")
                    nc.sync.dma_start(ut[:], uT[ts(t, P), ts(ic, IC)])
                    uts.append(ut)
                for vt in range(nUV):
                    ug_ps = ugps.tile([P, IC], F32)
                    for t in range(nD):
                        for k in range(icb):
                            nc.tensor.matmul(ug_ps[:, ts(k, LC)],
                                             wu_sb[t][:, ts(vt, P)],
                                             uts[t][:, ts(k, LC)],
                                             start=(t == 0), stop=(t == nD - 1))
                    ug_bf = ugsb.tile([P, IC], BF16, tag="ug")
                    nc.scalar.activation(ug_bf[:], ug_ps[:], AF.Gelu_apprx_tanh,
                                         bias=bu_sb[:, vt:vt + 1])
                    nc.sync.dma_start(ug_spill[vt, ic], ug_bf[:])

        with ExitStack() as vctx:
            vtp = vctx.enter_context(tc.tile_pool(name="vtp", bufs=1))
            vt_sb = [vtp.tile([P, L], F32R, name=f"vt_sb{t}") for t in range(nD)]
            for t in range(nD):
                nc.sync.dma_start(vt_sb[t][:], vT[ts(t, P), :].bitcast(F32R))
            wvp = vctx.enter_context(tc.tile_pool(name="wvp", bufs=2 * nD + 2))
            vps = vctx.enter_context(tc.tile_pool(name="vps", bufs=3, space="PSUM"))
            vsb = vctx.enter_context(tc.tile_pool(name="vsb", bufs=4))
            nVB = LC // P
            for vc in range(UV // LC):
                wvs = []
                for t in range(nD):
                    wv = wvp.tile([P, LC], F32R, tag="wv")
                    nc.sync.dma_start(wv[:], Wv[ts(t, P), ts(vc, LC)].bitcast(F32R))
                    wvs.append(wv)
                for jt in range(nL):
                    v_ps = vps.tile([P, LC], F32)
                    for t in range(nD):
                        nc.tensor.matmul(v_ps[:], vt_sb[t][:, ts(jt, P)], wvs[t][:],
                                         start=(t == 0), stop=False)
                    nc.tensor.matmul(v_ps[:], ones_sb[:], bv_sb[:, ts(vc, LC)],
                                     start=False, stop=True)
                    v_bf = vsb.tile([P, LC], BF16, tag="vbf")
                    nc.scalar.activation(v_bf[:], v_ps[:], AF.Gelu_apprx_tanh)
                    for k in range(nVB):
                        nc.sync.dma_start(v_spill[vc * nVB + k, :, ts(jt, P)],
                                          v_bf[:, ts(k, P)])

        # ================= Phase C/D: e_T, AV, out, o =====================
        with ExitStack() as cctx:
            wop = cctx.enter_context(tc.tile_pool(name="wop", bufs=1))
            wo_sb = [wop.tile([P, D], BF16, name=f"wo_sb{vt}") for vt in range(nUV)]
            for vt in range(nUV):
                nc.sync.dma_start(wo_sb[vt][:], Wo[ts(vt, P), :])
            stps = cctx.enter_context(tc.tile_pool(name="stps", bufs=1, space="PSUM"))
            ep = cctx.enter_context(tc.tile_pool(name="ep", bufs=nL + 2))
            vstr = cctx.enter_context(tc.tile_pool(name="vstr", bufs=3))
            ugstr = cctx.enter_context(tc.tile_pool(name="ugstr", bufs=4))
            avps = cctx.enter_context(tc.tile_pool(name="avps", bufs=2, space="PSUM"))
            outp = cctx.enter_context(tc.tile_pool(name="outp", bufs=nUV + 2))
            ops_ = cctx.enter_context(tc.tile_pool(name="ops", bufs=1, space="PSUM"))
            osb = cctx.enter_context(tc.tile_pool(name="osb", bufs=3))

            for ic in range(nIC):
                e_tiles = []
                for jt in range(nL):
                    st_ps = stps.tile([P, IC], F32)
                    for k in range(icb):
                        nc.tensor.matmul(st_ps[:, ts(k, LC)], k_rope[:, ts(jt, P)],
                                         q_rope[:, ts(ic * icb + k, LC)],
                                         start=True, stop=True)
                    e_bf = ep.tile([P, IC], BF16, tag="ebf")
                    nc.scalar.activation(e_bf[:], st_ps[:], AF.Exp)
                    e_tiles.append(e_bf)
                out_tiles = []
                for vt in range(nUV):
                    av_ps = avps.tile([P, IC], F32)
                    vslab = vstr.tile([P, L], BF16, tag="vslab")
                    nc.sync.dma_start(vslab[:], v_spill[vt])
                    for jt in range(nL):
                        for k in range(icb):
                            nc.tensor.matmul(av_ps[:, ts(k, LC)],
                                             vslab[:, ts(jt, P)],
                                             e_tiles[jt][:, ts(k, LC)],
                                             start=(jt == 0), stop=(jt == nL - 1))
                    ug_bf = ugstr.tile([P, IC], BF16, tag="ugs")
                    nc.sync.dma_start(ug_bf[:], ug_spill[vt, ic])
                    out_bf = outp.tile([P, IC], BF16, tag="out")
                    nc.vector.tensor_mul(out_bf[:], av_ps[:], ug_bf[:])
                    out_tiles.append(out_bf)
                for dt in range(nD):
                    o_ps = ops_.tile([P, IC], F32)
                    for vt in range(nUV):
                        for k in range(icb):
                            nc.tensor.matmul(o_ps[:, ts(k, LC)],
                                             wo_sb[vt][:, ts(dt, P)],
                                             out_tiles[vt][:, ts(k, LC)],
                                             start=(vt == 0), stop=(vt == nUV - 1))
                    o_sb = osb.tile([P, IC], F32, tag="osb")
                    nc.vector.tensor_mul(o_sb[:], o_ps[:], r_bcast[:, ts(ic, IC)])
                    nc.sync.dma_start(oT_out[ts(dt, P), ts(ic, IC)], o_sb[:])

    nc.compile()
    return nc


def prep_inputs(u, queries, keys, values, Wqk, bqk, gq, betq, gk, betk,
                Wv, bv, Wu, bu, Wo, bo, shared_qk, use_dq, use_dk):
    scale = float(QK) ** -0.25
    inv = 1.0 / (ROPE_BASE ** (np.arange(0, QK, 2, dtype=np.float64) / QK))
    t = np.arange(L, dtype=np.float64)
    f = t[None, :] * inv[:, None]            # [QK/2, L]
    c, s = np.cos(f), np.sin(f)

    def mk_tables(g, bet):
        g64, b64 = np.asarray(g, np.float64), np.asarray(bet, np.float64)
        C = np.empty((P, L))
        S = np.empty((P, L))
        Dt = np.empty((P, L))
        C[0::2] = g64[0::2, None] * c
        C[1::2] = g64[1::2, None] * c
        S[0::2] = -g64[1::2, None] * s
        S[1::2] = g64[0::2, None] * s
        Dt[0::2] = b64[0::2, None] * c - b64[1::2, None] * s
        Dt[1::2] = b64[0::2, None] * s + b64[1::2, None] * c
        return ((C * scale).astype(np.float32), (S * scale).astype(np.float32),
                (Dt * scale).astype(np.float32))

    CQ, SQ, DQ = mk_tables(gq, betq)
    PSW = np.zeros((P, P), np.float32)
    for i in range(0, P, 2):
        PSW[i, i + 1] = 1.0
        PSW[i + 1, i] = 1.0

    static = {
        "Wqk": np.ascontiguousarray(Wqk, np.float32),
        "Wv": np.ascontiguousarray(Wv, np.float32),
        "Wu": np.ascontiguousarray(Wu).astype(ml_dtypes.bfloat16),
        "Wo": np.ascontiguousarray(Wo).astype(ml_dtypes.bfloat16),
        "bqk": np.asarray(bqk, np.float32).reshape(P, 1),
        "bv": np.asarray(bv, np.float32).reshape(1, UV),
        "bu": np.ascontiguousarray(np.asarray(bu, np.float32).reshape(UV // P, P).T),
        "CQ": CQ, "SQ": SQ, "PSW": PSW,
        "ones1": np.ones((1, P), np.float32),
    }
    if use_dq:
        static["DQ"] = DQ
    if not shared_qk:
        CK, SK, DK = mk_tables(gk, betk)
        static["CK"], static["SK"] = CK, SK
        if use_dk:
            static["DK"] = DK

    in_maps = []
    for b in range(B):
        m = dict(static)
        m["qT"] = np.ascontiguousarray(queries[b].T, np.float32)
        m["vT"] = np.ascontiguousarray(values[b].T, np.float32)
        m["uT"] = np.ascontiguousarray(u[b].T).astype(ml_dtypes.bfloat16)
        in_maps.append(m)
    return in_maps


_cache = {}


def _get_program(flags):
    if flags not in _cache:
        _cache[flags] = build_program(*flags)
    return _cache[flags]


def kernel(u, queries, keys, values, Wqk, bqk, gq, betq, gk, betk,
           Wv, bv, Wu, bu, Wo, bo, _trace=False, _tmpdir=None):
    shared_qk = (np.array_equal(np.asarray(gq), np.asarray(gk))
                 and np.array_equal(np.asarray(betq), np.asarray(betk)))
    use_dq = bool(np.any(np.asarray(betq)))
    use_dk = (not shared_qk) and bool(np.any(np.asarray(betk)))

    nc = _get_program((shared_qk, use_dq, use_dk))
    in_maps = prep_inputs(u, queries, keys, values, Wqk, bqk, gq, betq, gk,
                          betk, Wv, bv, Wu, bu, Wo, bo,
                          shared_qk, use_dq, use_dk)
    res = bass_utils.run_bass_kernel_spmd(
        nc, in_maps, core_ids=list(range(N_CORES)),
        trace=_trace, tmpdir=_tmpdir)

    bo32 = np.asarray(bo, np.float32)
    o = np.empty((B, L, D), np.float32)
    attn = np.empty((B, L, L), np.float32)
    for b in range(B):
        o[b] = res.results[b]["oT"].T + bo32
        attn[b] = res.results[b]["attn"]
    if _trace:
        kernel._last_exec_time_ns = res.exec_time_ns
    return o, attn
